# revision 9
# baseline (speedup 1.0000x reference)
"""Trainium2 Bass kernel for CustomMamba single-token step.

Sharding: data-parallel over batch B=2048 across 8 cores (256 rows each).
Weights replicated. Returns (out, new_rnn_states) like the reference.

Per-core layouts:
  - matmul chain activations: [feature(partitions), batch(free)], bf16 on PE
  - SSM state update: [batch(partitions, 2 tiles of 128), (d, n) free], fp32
    on DVE with fused scalar_tensor_tensor ops; per-(batch) scalars Bm/Cm
    live as per-partition [128,1] columns.
rnn_states DRAM layout per row: (d_inner=2048, 20) with cols 0..3 = conv
window, 4..19 = ssm state. Read twice (conv pass + state pass), written once.
"""
import numpy as np
from contextlib import ExitStack

# ---- problem constants (hardcoded; kernel.py must be self-contained) ----
B = 2048
NCORES = 8
BC = B // NCORES          # 256 batch rows per core
P = 128                   # partitions
NBT = BC // P             # 2 batch tiles per core
DM = 1024                 # d_model
D = 2048                  # d_inner
NST = 16                  # d_state
DCV = 4                   # d_conv
DTR = 64                  # dt_rank
NIN = 512                 # input size
NOUT = 512                # output size
T = DCV + NST             # 20, rnn inner period
DBLK = D // P             # 16 d-blocks of 128
DC2 = 128                 # phase-2 d-chunk
NCH = D // DC2            # 8 chunks

_CACHE = {}


def _build():
    from concourse import bacc, mybir, tile, masks

    F32 = mybir.dt.float32
    BF16 = mybir.dt.bfloat16
    AF = mybir.ActivationFunctionType
    ALU = mybir.AluOpType

    nc = bacc.Bacc("TRN2", target_bir_lowering=False, debug=False)

    # ---- DRAM I/O (per-core shard for x/rnn; weights replicated) ----
    x_d = nc.dram_tensor("x_c", [BC, NIN], F32, kind="ExternalInput").ap()
    rnn_d = nc.dram_tensor("rnn_c", [BC, D * T], F32, kind="ExternalInput").ap()
    w_inp_d = nc.dram_tensor("w_inp", [NIN, DM], F32, kind="ExternalInput").ap()
    b_inp_d = nc.dram_tensor("b_inp", [DM], F32, kind="ExternalInput").ap()
    w_outp_d = nc.dram_tensor("w_outp", [DM, NOUT], F32, kind="ExternalInput").ap()
    b_outp_d = nc.dram_tensor("b_outp", [NOUT], F32, kind="ExternalInput").ap()
    inproj_d = nc.dram_tensor("in_proj_w", [2 * D, DM], F32, kind="ExternalInput").ap()
    convw_d = nc.dram_tensor("conv_w", [D, DCV], F32, kind="ExternalInput").ap()
    convb_d = nc.dram_tensor("conv_b", [D], F32, kind="ExternalInput").ap()
    xproj_d = nc.dram_tensor("x_proj_w", [DTR + 2 * NST, D], F32, kind="ExternalInput").ap()
    dtw_d = nc.dram_tensor("dt_proj_w", [D, DTR], F32, kind="ExternalInput").ap()
    dtb_d = nc.dram_tensor("dt_proj_b", [D], F32, kind="ExternalInput").ap()
    alog_d = nc.dram_tensor("A_log", [D, NST], F32, kind="ExternalInput").ap()
    dvec_d = nc.dram_tensor("D_vec", [D], F32, kind="ExternalInput").ap()
    outproj_d = nc.dram_tensor("out_proj_w", [DM, D], F32, kind="ExternalInput").ap()

    out_d = nc.dram_tensor("out_c", [BC, NOUT], F32, kind="ExternalOutput").ap()
    rnnout_d = nc.dram_tensor("rnn_out_c", [BC, D * T], F32, kind="ExternalOutput").ap()

    with tile.TileContext(nc) as tc, ExitStack() as top:
        const = top.enter_context(tc.tile_pool(name="const", bufs=1))
        pers = top.enter_context(tc.tile_pool(name="pers", bufs=1))
        ps_mm = top.enter_context(tc.tile_pool(name="ps_mm", bufs=3, space="PSUM"))
        ps_tr = top.enter_context(tc.tile_pool(name="ps_tr", bufs=3, space="PSUM"))
        ps_xdb = top.enter_context(tc.tile_pool(name="ps_xdb", bufs=1, space="PSUM"))

        # ---- constants / small vectors ----
        ident = const.tile([P, P], F32)
        masks.make_identity(nc, ident[:])
        ident_bf = const.tile([P, P], BF16)
        masks.make_identity(nc, ident_bf[:])

        # all small per-partition vectors packed into one tile (avoids 4KB
        # per-tile padding): cols 0:128 ones-row, 128:136 b_inp, 136:140
        # b_outp, 140:156 conv_b, 156:172 dt_proj_b, 172:188 D, 192:256 conv_w
        pack = const.tile([P, 256], F32)
        ones_f = pack[0:1, 0:P]
        nc.vector.memset(ones_f, 1.0)
        b_inp = pack[:, 128:136]
        nc.sync.dma_start(b_inp, b_inp_d.rearrange("(m p) -> p m", p=P))
        b_outp = pack[:, 136:140]
        nc.sync.dma_start(b_outp, b_outp_d.rearrange("(m p) -> p m", p=P))
        conv_b = pack[:, 140:156]
        nc.sync.dma_start(conv_b, convb_d.rearrange("(t p) -> p t", p=P))
        dtb_col = pack[:, 156:172]
        nc.sync.dma_start(dtb_col, dtb_d.rearrange("(t p) -> p t", p=P))
        d_vec = pack[:, 172:188]
        nc.sync.dma_start(d_vec, dvec_d.rearrange("(t p) -> p t", p=P))
        conv_w = pack[:, 192:256]
        nc.sync.dma_start(
            conv_w.rearrange("p (t j) -> p t j", j=DCV),
            convw_d.rearrange("(t p) j -> p t j", p=P),
        )

        # ---- persistent activations (live into phase 2/3) ----
        xiT = pers.tile([P, DBLK * BC], BF16)          # xi in [d, b]
        szT = pers.tile([P, DBLK * BC], BF16)          # silu(z) [d, b]
        xcT = pers.tile([P, DBLK * BC], BF16)          # silu conv out [d, b]
        dt_b = [pers.tile([P, D], F32, name=f"dt_b{t}", tag=f"dt_b{t}") for t in range(NBT)]
        dtxc_b = [pers.tile([P, D], BF16, name=f"dtxc_b{t}", tag=f"dtxc_b{t}") for t in range(NBT)]
        y_b = [pers.tile([P, D], F32, name=f"y_b{t}", tag=f"y_b{t}") for t in range(NBT)]
        bmcm = [pers.tile([P, 2 * NST], F32, name=f"bmcm{t}", tag=f"bmcm{t}") for t in range(NBT)]

        # ================= phase 0/1: weights, matmul chain, conv =================
        with ExitStack() as ph0:
            # small weights that must survive until xdb/dt matmuls
            wkeep = ph0.enter_context(tc.tile_pool(name="wkeep", bufs=1))
            xpT = wkeep.tile([P, 16 * 96], BF16)
            dtpT = wkeep.tile([DTR, D], BF16)
            xdb_dt = wkeep.tile([DTR, BC], BF16)
            bmcm_sb = wkeep.tile([2 * NST, BC], F32)
            xiT_holder = []

            with ExitStack() as ph0a:
                wload = ph0a.enter_context(tc.tile_pool(name="wload", bufs=3))
                wtmp = ph0a.enter_context(tc.tile_pool(name="wtmp", bufs=1))

                # w_inp natural [K=512, M=1024] -> bf16
                w_inp = wtmp.tile([P, 4 * DM], BF16)  # 4 k-tiles side by side
                for k in range(4):
                    t = wload.tile([P, DM], F32, tag="w")
                    nc.sync.dma_start(t[:], w_inp_d[k * P:(k + 1) * P, :])
                    nc.vector.tensor_copy(w_inp[:, k * DM:(k + 1) * DM], t[:])

                # xT: transpose x [256, 512] -> [512(4k), 256] bf16
                xT = wtmp.tile([P, 4 * BC], BF16)
                for bt in range(NBT):
                    t = wload.tile([P, NIN], F32, tag="w")
                    nc.sync.dma_start(t[:], x_d[bt * P:(bt + 1) * P, :])
                    for k in range(4):
                        pt = ps_tr.tile([P, 512], F32, tag="tr")
                        nc.tensor.transpose(
                            pt[:, 0:P], t[:, k * P:(k + 1) * P], ident[:]
                        )
                        nc.scalar.copy(
                            xT[:, k * BC + bt * P: k * BC + (bt + 1) * P], pt[:, 0:P]
                        )

                # in_projT: [1024(8k), 4096] bf16, transposed from (4096, 1024)
                inT = wtmp.tile([P, 8 * 2 * D], BF16)
                for rg in range(8):  # row-groups of 4 tiles
                    tiles = []
                    for rr in range(4):
                        t = wload.tile([P, DM], F32, tag="wip", bufs=5)
                        nc.sync.dma_start(
                            t[:], inproj_d[(rg * 4 + rr) * P:(rg * 4 + rr + 1) * P, :]
                        )
                        tiles.append(t)
                    for k in range(8):
                        pt = ps_tr.tile([P, 512], F32, tag="tr")
                        for rr in range(4):
                            nc.tensor.transpose(
                                pt[:, rr * P:(rr + 1) * P],
                                tiles[rr][:, k * P:(k + 1) * P], ident[:]
                            )
                        dst = inT[:, k * 2 * D + rg * 512: k * 2 * D + (rg + 1) * 512]
                        if k % 2 == 0:
                            nc.scalar.copy(dst, pt[:])
                        else:
                            nc.vector.tensor_copy(dst, pt[:])

                # x_projT: [2048(16k), 96] bf16 from (96, 2048)
                txp = wload.tile([96, D], F32, tag="w2", bufs=1)
                nc.sync.dma_start(txp[:], xproj_d)
                for k in range(16):
                    pt = ps_tr.tile([P, 512], F32, tag="tr")
                    nc.tensor.transpose(
                        pt[:, 0:96], txp[:, k * P:(k + 1) * P], ident[0:96, 0:96]
                    )
                    nc.scalar.copy(xpT[:, k * 96:(k + 1) * 96], pt[:, 0:96])

                # dt_projT: [64, 2048] bf16 from (2048, 64)
                for k in range(16):
                    t = wload.tile([P, DTR], F32, tag="w")
                    nc.sync.dma_start(t[:], dtw_d[k * P:(k + 1) * P, :])
                    pt = ps_tr.tile([P, 512], F32, tag="tr")
                    nc.tensor.transpose(pt[0:DTR, 0:P], t[:], ident[:])
                    nc.scalar.copy(dtpT[:, k * P:(k + 1) * P], pt[0:DTR, 0:P])

                # ---- h = x @ w_inp + b_inp, in [feat, b] bf16 ----
                hT = wtmp.tile([P, (DM // P) * BC], BF16)
                for m in range(DM // P):  # 8
                    pm = ps_mm.tile([P, BC], F32, tag="mm")
                    for k in range(4):
                        nc.tensor.matmul(
                            pm[:], w_inp[:, k * DM + m * P: k * DM + (m + 1) * P],
                            xT[:, k * BC:(k + 1) * BC],
                            start=(k == 0), stop=(k == 3),
                        )
                    nc.scalar.activation(
                        hT[:, m * BC:(m + 1) * BC], pm[:], AF.Identity,
                        bias=b_inp[:, m:m + 1], scale=1.0,
                    )

                # ---- xz = h @ in_proj.T -> xiT (m<16), szT=silu(z) ----
                for m in range(32):
                    pm = ps_mm.tile([P, BC], F32, tag="mm")
                    for k in range(8):
                        nc.tensor.matmul(
                            pm[:], inT[:, k * 2 * D + m * P: k * 2 * D + (m + 1) * P],
                            hT[:, k * BC:(k + 1) * BC],
                            start=(k == 0), stop=(k == 7),
                        )
                    if m < 16:
                        nc.vector.tensor_copy(xiT[:, m * BC:(m + 1) * BC], pm[:])
                    else:
                        mm = m - 16
                        nc.scalar.activation(
                            szT[:, mm * BC:(mm + 1) * BC], pm[:], AF.Silu,
                            bias=0.0, scale=1.0,
                        )

            # ---- phase 1: conv over rnn (first read) + xdb accumulation ----
            with ExitStack() as ph0b:
                p1 = ph0b.enter_context(tc.tile_pool(name="p1", bufs=3))
                wdt = ph0b.enter_context(tc.tile_pool(name="wdt", bufs=1))

                pxdb = ps_xdb.tile([96, BC], F32)
                cwv = conv_w.rearrange("p (t j) -> p t j", j=DCV)
                for dblk in range(DBLK):
                    for bt in range(NBT):
                        in1 = p1.tile([P, P * T], F32, tag="in1")
                        nc.sync.dma_start(
                            in1[:],
                            rnn_d[bt * P:(bt + 1) * P,
                                  dblk * P * T:(dblk + 1) * P * T],
                        )
                        iv = in1[:].rearrange("p (d t) -> p d t", t=T)
                        # transpose cs_1..3 -> [d, b] into one psum tile
                        pt = ps_tr.tile([P, 512], F32, tag="tr")
                        for j in (1, 2, 3):
                            nc.tensor.transpose(
                                pt[:, (j - 1) * P: j * P], iv[:, :, j], ident[:]
                            )
                        tmp = p1.tile([P, P], F32, tag="cv")
                        nc.vector.tensor_scalar_mul(
                            tmp[:], pt[:, 0:P], cwv[:, dblk, 0:1]
                        )
                        nc.vector.scalar_tensor_tensor(
                            tmp[:], pt[:, P:2 * P], cwv[:, dblk, 1:2], tmp[:],
                            op0=ALU.mult, op1=ALU.add,
                        )
                        nc.vector.scalar_tensor_tensor(
                            tmp[:], pt[:, 2 * P:3 * P], cwv[:, dblk, 2:3], tmp[:],
                            op0=ALU.mult, op1=ALU.add,
                        )
                        nc.vector.scalar_tensor_tensor(
                            tmp[:],
                            xiT[:, dblk * BC + bt * P: dblk * BC + (bt + 1) * P],
                            cwv[:, dblk, 3:4], tmp[:], op0=ALU.mult, op1=ALU.add,
                        )
                        nc.scalar.activation(
                            xcT[:, dblk * BC + bt * P: dblk * BC + (bt + 1) * P],
                            tmp[:], AF.Silu, bias=conv_b[:, dblk:dblk + 1],
                            scale=1.0,
                        )
                    # accumulate xdb += xpT[dblk].T @ xcT[dblk]
                    nc.tensor.matmul(
                        pxdb[:], xpT[:, dblk * 96:(dblk + 1) * 96],
                        xcT[:, dblk * BC:(dblk + 1) * BC],
                        start=(dblk == 0), stop=(dblk == DBLK - 1),
                    )

                # ---- xdb split: dt-rank rows + Bm/Cm transposed per btile ----
                nc.vector.tensor_copy(xdb_dt[:], pxdb[0:DTR, :])
                nc.vector.tensor_copy(bmcm_sb[:], pxdb[DTR:96, :])
                for bt in range(NBT):
                    pt = ps_tr.tile([P, 512], F32, tag="tr")
                    nc.tensor.transpose(
                        pt[:, 0:2 * NST], bmcm_sb[:, bt * P:(bt + 1) * P],
                        ident[0:2 * NST, 0:2 * NST],
                    )
                    nc.vector.tensor_copy(bmcm[bt][:], pt[:, 0:2 * NST])

                # ---- dt = softplus(xdb_dt @ dt_proj.T + dt_proj_b) ----
                dtT = wdt.tile([P, DBLK * BC], F32)
                dtxcT = wdt.tile([P, DBLK * BC], F32)
                for m in range(DBLK):
                    pm = ps_mm.tile([P, BC], F32, tag="mm")
                    nc.tensor.matmul(
                        pm[:], dtpT[:, m * P:(m + 1) * P], xdb_dt[:],
                        start=True, stop=True,
                    )
                    te = p1.tile([P, BC], F32, tag="sp")
                    nc.scalar.activation(
                        te[:], pm[:], AF.Exp, bias=dtb_col[:, m:m + 1], scale=1.0
                    )
                    nc.scalar.activation(
                        dtT[:, m * BC:(m + 1) * BC], te[:], AF.Ln,
                        bias=1.0, scale=1.0,
                    )
                # dtxcT = dt * xc
                nc.vector.tensor_tensor(dtxcT[:], dtT[:], xcT[:], op=ALU.mult)

                # ---- transpose dtT, dtxcT to [b, d] (groups of 4 dblks) ----
                for bt in range(NBT):
                    for g in range(4):
                        pt = ps_tr.tile([P, 512], F32, tag="tr")
                        pt2 = ps_tr.tile([P, 512], F32, tag="tr")
                        for i in range(4):
                            dblk = g * 4 + i
                            nc.tensor.transpose(
                                pt[:, i * P:(i + 1) * P],
                                dtT[:, dblk * BC + bt * P: dblk * BC + (bt + 1) * P],
                                ident[:],
                            )
                            nc.tensor.transpose(
                                pt2[:, i * P:(i + 1) * P],
                                dtxcT[:, dblk * BC + bt * P: dblk * BC + (bt + 1) * P],
                                ident[:],
                            )
                        nc.vector.tensor_copy(
                            dt_b[bt][:, g * 512:(g + 1) * 512], pt[:]
                        )
                        nc.scalar.copy(
                            dtxc_b[bt][:, g * 512:(g + 1) * 512], pt2[:]
                        )

        # ================= phase 2: SSM state update =================
        with ExitStack() as ph2:
            p2 = ph2.enter_context(tc.tile_pool(name="p2", bufs=2))
            p2o = ph2.enter_context(tc.tile_pool(name="p2o", bufs=2))
            p2t = ph2.enter_context(tc.tile_pool(name="p2t", bufs=3))
            p2da = ph2.enter_context(tc.tile_pool(name="p2da", bufs=2))
            p2a = ph2.enter_context(tc.tile_pool(name="p2a", bufs=2))
            parow = ph2.enter_context(tc.tile_pool(name="parow", bufs=2))
            alog_flat = alog_d.rearrange("d n -> (d n)")

            for ch in range(NCH):
                # A_rep = exp(A_log) replicated across partitions, bf16
                arow_t = parow.tile([1, DC2 * NST], F32, tag="ar")
                nc.sync.dma_start(
                    arow_t[:],
                    alog_flat[ch * DC2 * NST:(ch + 1) * DC2 * NST].unsqueeze(0),
                )
                a_rep = p2a.tile([P, DC2 * NST], BF16, tag="arep")
                for g in range(DC2 * NST // 512):  # 4
                    pa = ps_tr.tile([P, 512], F32, tag="tr")
                    nc.tensor.matmul(
                        pa[:], ones_f, arow_t[:, g * 512:(g + 1) * 512],
                        start=True, stop=True,
                    )
                    nc.scalar.activation(
                        a_rep[:, g * 512:(g + 1) * 512], pa[:], AF.Exp,
                        bias=0.0, scale=1.0,
                    )
                a_rep_v = a_rep[:].rearrange("p (d n) -> p d n", n=NST)

                for bt in range(NBT):
                    in2 = p2.tile([P, DC2 * T], F32, tag="in2")
                    nc.sync.dma_start(
                        in2[:],
                        rnn_d[bt * P:(bt + 1) * P, ch * DC2 * T:(ch + 1) * DC2 * T],
                    )
                    out2 = p2o.tile([P, DC2 * T], F32, tag="out2")
                    iv = in2[:].rearrange("p (d t) -> p d t", t=T)
                    ov = out2[:].rearrange("p (d t) -> p d t", t=T)

                    dts = dt_b[bt][:, ch * DC2:(ch + 1) * DC2]
                    dts_b = dts.unsqueeze(2).broadcast_to([P, DC2, NST])

                    prod = p2t.tile([P, DC2 * NST], F32, tag="t")
                    pv = prod[:].rearrange("p (d n) -> p d n", n=NST)
                    nc.vector.tensor_tensor(pv, dts_b, a_rep_v, op=ALU.mult)

                    dA = p2da.tile([P, DC2 * NST], BF16, tag="da")
                    nc.scalar.activation(dA[:], prod[:], AF.Exp, bias=0.0, scale=-1.0)

                    m = p2t.tile([P, DC2 * NST], F32, tag="t")
                    mv = m[:].rearrange("p (d n) -> p d n", n=NST)
                    dAv = dA[:].rearrange("p (d n) -> p d n", n=NST)
                    nc.vector.tensor_tensor(mv, iv[:, :, DCV:T], dAv, op=ALU.mult)

                    # conv window shift: out cols 0..2 = in cols 1..3
                    nc.vector.tensor_copy(ov[:, :, 0:DCV - 1], iv[:, :, 1:DCV])
                    # out col 3 = xi (transposed from xiT for this d-block)
                    pt = ps_tr.tile([P, 512], BF16, name="ptxi", tag="tr")
                    nc.tensor.transpose(
                        pt[:, 0:P],
                        xiT[:, ch * BC + bt * P: ch * BC + (bt + 1) * P],
                        ident_bf[:],
                    )
                    nc.vector.tensor_copy(ov[:, :, DCV - 1], pt[:, 0:P])

                    dtxcs = dtxc_b[bt][:, ch * DC2:(ch + 1) * DC2]
                    ys = y_b[bt][:, ch * DC2:(ch + 1) * DC2]
                    for n in range(NST):
                        nc.vector.scalar_tensor_tensor(
                            ov[:, :, DCV + n], dtxcs, bmcm[bt][:, n:n + 1],
                            mv[:, :, n], op0=ALU.mult, op1=ALU.add,
                        )
                    for n in range(NST):
                        if n == 0:
                            nc.vector.tensor_scalar_mul(
                                ys, ov[:, :, DCV], bmcm[bt][:, NST:NST + 1]
                            )
                        else:
                            nc.vector.scalar_tensor_tensor(
                                ys, ov[:, :, DCV + n], bmcm[bt][:, NST + n:NST + n + 1],
                                ys, op0=ALU.mult, op1=ALU.add,
                            )
                    nc.sync.dma_start(
                        rnnout_d[bt * P:(bt + 1) * P, ch * DC2 * T:(ch + 1) * DC2 * T],
                        out2[:],
                    )

        # ================= phase 3: y -> out =================
        with ExitStack() as ph3:
            p3 = ph3.enter_context(tc.tile_pool(name="p3", bufs=1))
            p3l = ph3.enter_context(tc.tile_pool(name="p3l", bufs=3))

            # out_projT: [2048(16k), 1024] bf16 from (1024, 2048)
            opT = p3.tile([P, 16 * DM], BF16)
            for rg in range(2):
                tiles = []
                for rr in range(4):
                    t = p3l.tile([P, D], F32, tag="wop", bufs=5)
                    nc.sync.dma_start(
                        t[:], outproj_d[(rg * 4 + rr) * P:(rg * 4 + rr + 1) * P, :]
                    )
                    tiles.append(t)
                for k in range(16):
                    pt = ps_tr.tile([P, 512], F32, tag="tr")
                    for rr in range(4):
                        nc.tensor.transpose(
                            pt[:, rr * P:(rr + 1) * P],
                            tiles[rr][:, k * P:(k + 1) * P], ident[:]
                        )
                    dst = opT[:, k * DM + rg * 512: k * DM + (rg + 1) * 512]
                    if k % 2 == 0:
                        nc.scalar.copy(dst, pt[:])
                    else:
                        nc.vector.tensor_copy(dst, pt[:])

            # w_outp natural [K=1024, M=512] -> bf16
            w_outp = p3.tile([P, 8 * NOUT], BF16)
            for k in range(8):
                t = p3l.tile([P, NOUT], F32, tag="w2")
                nc.sync.dma_start(t[:], w_outp_d[k * P:(k + 1) * P, :])
                nc.vector.tensor_copy(w_outp[:, k * NOUT:(k + 1) * NOUT], t[:])

            # yT = y.T + D*xc  (per d-block), then yfT = yT * silu(z)
            yT = p3.tile([P, DBLK * BC], F32)
            for bt in range(NBT):
                for dblk in range(DBLK):
                    pt = ps_tr.tile([P, 512], F32, tag="tr")
                    nc.tensor.transpose(
                        pt[:, 0:P], y_b[bt][:, dblk * P:(dblk + 1) * P], ident[:]
                    )
                    nc.vector.scalar_tensor_tensor(
                        yT[:, dblk * BC + bt * P: dblk * BC + (bt + 1) * P],
                        xcT[:, dblk * BC + bt * P: dblk * BC + (bt + 1) * P],
                        d_vec[:, dblk:dblk + 1], pt[:, 0:P],
                        op0=ALU.mult, op1=ALU.add,
                    )
            yfT = p3.tile([P, DBLK * BC], BF16)
            nc.vector.tensor_tensor(yfT[:], yT[:], szT[:], op=ALU.mult)

            # core_out = yf @ out_proj.T -> [1024(8m), b] bf16
            coT = p3.tile([P, 8 * BC], BF16)
            for m in range(8):
                pm = ps_mm.tile([P, BC], F32, tag="mm")
                for k in range(16):
                    nc.tensor.matmul(
                        pm[:], opT[:, k * DM + m * P: k * DM + (m + 1) * P],
                        yfT[:, k * BC:(k + 1) * BC],
                        start=(k == 0), stop=(k == 15),
                    )
                nc.scalar.copy(coT[:, m * BC:(m + 1) * BC], pm[:])

            # out = core_out @ w_outp + b_outp -> [512(4m), b] f32
            outT = p3.tile([P, 4 * BC], F32)
            for m in range(4):
                pm = ps_mm.tile([P, BC], F32, tag="mm")
                for k in range(8):
                    nc.tensor.matmul(
                        pm[:], w_outp[:, k * NOUT + m * P: k * NOUT + (m + 1) * P],
                        coT[:, k * BC:(k + 1) * BC],
                        start=(k == 0), stop=(k == 7),
                    )
                nc.scalar.activation(
                    outT[:, m * BC:(m + 1) * BC], pm[:], AF.Identity,
                    bias=b_outp[:, m:m + 1], scale=1.0,
                )

            # transpose outT -> [b, 512] and store
            for bt in range(NBT):
                ob = p3l.tile([P, NOUT], F32, tag="ob")
                pt = ps_tr.tile([P, 512], F32, tag="tr")
                for m in range(4):
                    nc.tensor.transpose(
                        pt[:, m * P:(m + 1) * P],
                        outT[:, m * BC + bt * P: m * BC + (bt + 1) * P], ident[:]
                    )
                nc.vector.tensor_copy(ob[:], pt[:])
                nc.sync.dma_start(out_d[bt * P:(bt + 1) * P, :], ob[:])

    nc.compile()
    return nc


def _get_program():
    if "nc" not in _CACHE:
        _CACHE["nc"] = _build()
    return _CACHE["nc"]


def kernel(**inputs):
    from concourse.bass_utils import run_bass_kernel_spmd

    nc = _get_program()

    x = np.asarray(inputs["x"], dtype=np.float32)             # (1, B, 512)
    rnn = np.asarray(inputs["rnn_states"], dtype=np.float32)  # (1, B, 40960)
    weights = {
        "w_inp": inputs["w_inp"], "b_inp": inputs["b_inp"],
        "w_outp": inputs["w_outp"], "b_outp": inputs["b_outp"],
        "in_proj_w": inputs["in_proj_w"], "conv_w": inputs["conv_w"],
        "conv_b": inputs["conv_b"], "x_proj_w": inputs["x_proj_w"],
        "dt_proj_w": inputs["dt_proj_w"], "dt_proj_b": inputs["dt_proj_b"],
        "A_log": inputs["A_log"], "D_vec": inputs["D"],
        "out_proj_w": inputs["out_proj_w"],
    }
    weights = {k: np.ascontiguousarray(np.asarray(v, dtype=np.float32))
               for k, v in weights.items()}

    in_maps = []
    for c in range(NCORES):
        sl = slice(c * BC, (c + 1) * BC)
        m = dict(weights)
        m["x_c"] = np.ascontiguousarray(x[0, sl])
        m["rnn_c"] = np.ascontiguousarray(rnn[0, sl])
        in_maps.append(m)

    res = run_bass_kernel_spmd(nc, in_maps, list(range(NCORES))).results

    out = np.empty((1, B, NOUT), dtype=np.float32)
    new_rnn = np.empty((B, D * T), dtype=np.float32)
    for c in range(NCORES):
        sl = slice(c * BC, (c + 1) * BC)
        out[0, sl] = res[c]["out_c"]
        new_rnn[sl] = res[c]["rnn_out_c"]
    return out, new_rnn


# revision 26
# speedup vs baseline: 70.1430x; 70.1430x over previous
"""Trainium2 Bass kernel for CustomMamba single-token step.

Sharding: data-parallel over batch B=2048 across 8 cores (256 rows each).
Weights replicated. Returns (out, new_rnn_states) like the reference.

Per-core layouts:
  - matmul chain activations: [feature(partitions), batch(free)], bf16 on PE
  - SSM state update: [batch(partitions, 2 tiles of 128), (d, n) free], fp32
    on DVE with fused scalar_tensor_tensor ops; per-(batch) scalars Bm/Cm
    live as per-partition [128,1] columns.
rnn_states DRAM layout per row: (d_inner=2048, 20) with cols 0..3 = conv
window, 4..19 = ssm state. Read twice (conv pass + state pass), written once.
"""
import numpy as np
from contextlib import ExitStack

# ---- problem constants (hardcoded; kernel.py must be self-contained) ----
B = 2048
NCORES = 8
BC = B // NCORES          # 256 batch rows per core
P = 128                   # partitions
NBT = BC // P             # 2 batch tiles per core
DM = 1024                 # d_model
D = 2048                  # d_inner
NST = 16                  # d_state
DCV = 4                   # d_conv
DTR = 64                  # dt_rank
NIN = 512                 # input size
NOUT = 512                # output size
T = DCV + NST             # 20, rnn inner period
DBLK = D // P             # 16 d-blocks of 128
DC2 = 128                 # phase-2 d-chunk
NCH = D // DC2            # 8 chunks

_CACHE = {}


_POOL_MODE = 0  # 0: none, 1: prod all pool, 2: prod half pool, 3: m pool


def _build():
    from concourse import bacc, mybir, tile, masks

    F32 = mybir.dt.float32
    BF16 = mybir.dt.bfloat16
    AF = mybir.ActivationFunctionType
    ALU = mybir.AluOpType

    nc = bacc.Bacc("TRN2", target_bir_lowering=False, debug=False)

    # ---- DRAM I/O (per-core shard for x/rnn; weights replicated) ----
    x_d = nc.dram_tensor("x_c", [BC, NIN], F32, kind="ExternalInput").ap()
    rnn_d = nc.dram_tensor("rnn_c", [BC, D * T], F32, kind="ExternalInput").ap()
    w_inp_d = nc.dram_tensor("w_inp", [NIN, DM], F32, kind="ExternalInput").ap()
    b_inp_d = nc.dram_tensor("b_inp", [DM], F32, kind="ExternalInput").ap()
    w_outp_d = nc.dram_tensor("w_outp", [DM, NOUT], F32, kind="ExternalInput").ap()
    b_outp_d = nc.dram_tensor("b_outp", [NOUT], F32, kind="ExternalInput").ap()
    inproj_d = nc.dram_tensor("in_proj_w", [2 * D, DM], F32, kind="ExternalInput").ap()
    convw_d = nc.dram_tensor("conv_w", [D, DCV], F32, kind="ExternalInput").ap()
    convb_d = nc.dram_tensor("conv_b", [D], F32, kind="ExternalInput").ap()
    xproj_d = nc.dram_tensor("x_proj_w", [DTR + 2 * NST, D], F32, kind="ExternalInput").ap()
    dtw_d = nc.dram_tensor("dt_proj_w", [D, DTR], F32, kind="ExternalInput").ap()
    dtb_d = nc.dram_tensor("dt_proj_b", [D], F32, kind="ExternalInput").ap()
    alog_d = nc.dram_tensor("A_log", [D, NST], F32, kind="ExternalInput").ap()
    dvec_d = nc.dram_tensor("D_vec", [D], F32, kind="ExternalInput").ap()
    outproj_d = nc.dram_tensor("out_proj_w", [DM, D], F32, kind="ExternalInput").ap()

    out_d = nc.dram_tensor("out_c", [BC, NOUT], F32, kind="ExternalOutput").ap()
    rnnout_d = nc.dram_tensor("rnn_out_c", [BC, D * T], F32, kind="ExternalOutput").ap()

    with tile.TileContext(nc) as tc, ExitStack() as top:
        const = top.enter_context(tc.tile_pool(name="const", bufs=1))
        pers = top.enter_context(tc.tile_pool(name="pers", bufs=1))
        ps_mm = top.enter_context(tc.tile_pool(name="ps_mm", bufs=3, space="PSUM"))
        ps_tr = top.enter_context(tc.tile_pool(name="ps_tr", bufs=3, space="PSUM"))
        ps_xdb = top.enter_context(tc.tile_pool(name="ps_xdb", bufs=1, space="PSUM"))

        # ---- constants / small vectors ----
        ident = const.tile([P, P], F32)
        masks.make_identity(nc, ident[:])
        ident_bf = const.tile([P, P], BF16)
        masks.make_identity(nc, ident_bf[:])
        # A_log flat (d-major) viewed [16, 2048]; replicated to 128 partitions
        # per chunk via selector matmul (lhsT = identity column broadcast)
        arow16 = const.tile([NST, D], F32)
        nc.sync.dma_start(
            arow16[:], alog_d.rearrange("(r x) n -> r (x n)", x=P)
        )

        # all small per-partition vectors packed into one tile (avoids 4KB
        # per-tile padding): cols 0:128 ones-row, 128:136 b_inp, 136:140
        # b_outp, 140:156 conv_b, 156:172 dt_proj_b, 172:188 D, 192:256 conv_w
        pack = const.tile([P, 256], F32)
        ones_f = pack[0:1, 0:P]
        nc.vector.memset(ones_f, 1.0)
        b_inp = pack[:, 128:136]
        nc.sync.dma_start(b_inp, b_inp_d.rearrange("(m p) -> p m", p=P))
        b_outp = pack[:, 136:140]
        nc.sync.dma_start(b_outp, b_outp_d.rearrange("(m p) -> p m", p=P))
        conv_b = pack[:, 140:156]
        nc.sync.dma_start(conv_b, convb_d.rearrange("(t p) -> p t", p=P))
        dtb_col = pack[:, 156:172]
        nc.sync.dma_start(dtb_col, dtb_d.rearrange("(t p) -> p t", p=P))
        d_vec = pack[:, 172:188]
        nc.sync.dma_start(d_vec, dvec_d.rearrange("(t p) -> p t", p=P))
        conv_w = pack[:, 192:256]
        nc.sync.dma_start(
            conv_w.rearrange("p (t j) -> p t j", j=DCV),
            convw_d.rearrange("(t p) j -> p t j", p=P),
        )

        # ---- persistent activations (live into phase 2/3) ----
        xiT = pers.tile([P, DBLK * BC], BF16)          # xi in [d, b]
        szT = pers.tile([P, DBLK * BC], BF16)          # silu(z) [d, b]
        xcT = pers.tile([P, DBLK * BC], BF16)          # silu conv out [d, b]
        dt_b = [pers.tile([P, D], F32, name=f"dt_b{t}", tag=f"dt_b{t}") for t in range(NBT)]
        dtxc_b = [pers.tile([P, D], BF16, name=f"dtxc_b{t}", tag=f"dtxc_b{t}") for t in range(NBT)]
        y_b = [pers.tile([P, D], F32, name=f"y_b{t}", tag=f"y_b{t}") for t in range(NBT)]
        bmcm_t = pers.tile([P, 4 * NST], F32)
        bmcm = [bmcm_t[:, t * 2 * NST:(t + 1) * 2 * NST] for t in range(NBT)]

        # ================= phase 0/1: weights, matmul chain, conv =================
        with ExitStack() as ph0:
            # small weights that must survive until xdb/dt matmuls
            wkeep = ph0.enter_context(tc.tile_pool(name="wkeep", bufs=1))
            xpT = wkeep.tile([P, 16 * 96], BF16)
            dtpT = wkeep.tile([DTR, D], BF16)
            xdb_dt = wkeep.tile([DTR, BC], BF16)
            bmcm_sb = wkeep.tile([2 * NST, BC], F32)
            xiT_holder = []

            with ExitStack() as ph0a:
                wload = ph0a.enter_context(tc.tile_pool(name="wload", bufs=3))
                wtmp = ph0a.enter_context(tc.tile_pool(name="wtmp", bufs=1))

                # w_inp natural [K=512, M=1024] -> bf16
                w_inp = wtmp.tile([P, 4 * DM], BF16)  # 4 k-tiles side by side
                for k in range(4):
                    t = wload.tile([P, DM], F32, tag="w")
                    nc.sync.dma_start(t[:], w_inp_d[k * P:(k + 1) * P, :])
                    nc.vector.tensor_copy(w_inp[:, k * DM:(k + 1) * DM], t[:])

                # xT: transpose x [256, 512] -> [512(4k), 256] bf16
                xT = wtmp.tile([P, 4 * BC], BF16)
                for bt in range(NBT):
                    t = wload.tile([P, NIN], F32, tag="w")
                    nc.sync.dma_start(t[:], x_d[bt * P:(bt + 1) * P, :])
                    for k in range(4):
                        pt = ps_tr.tile([P, 512], F32, tag="tr")
                        nc.tensor.transpose(
                            pt[:, 0:P], t[:, k * P:(k + 1) * P], ident[:]
                        )
                        nc.scalar.copy(
                            xT[:, k * BC + bt * P: k * BC + (bt + 1) * P], pt[:, 0:P]
                        )

                # in_projT: [1024(8k), 4096] bf16, transposed from (4096, 1024)
                inT = wtmp.tile([P, 8 * 2 * D], BF16)
                for rg in range(8):  # row-groups of 4 tiles
                    tiles = []
                    for rr in range(4):
                        t = wload.tile([P, DM], F32, tag="wip", bufs=6)
                        nc.sync.dma_start(
                            t[:], inproj_d[(rg * 4 + rr) * P:(rg * 4 + rr + 1) * P, :]
                        )
                        tiles.append(t)
                    for k in range(8):
                        pt = ps_tr.tile([P, 512], F32, tag="tr")
                        for rr in range(4):
                            nc.tensor.transpose(
                                pt[:, rr * P:(rr + 1) * P],
                                tiles[rr][:, k * P:(k + 1) * P], ident[:]
                            )
                        dst = inT[:, k * 2 * D + rg * 512: k * 2 * D + (rg + 1) * 512]
                        if k % 2 == 0:
                            nc.scalar.copy(dst, pt[:])
                        else:
                            nc.vector.tensor_copy(dst, pt[:])

                # x_projT: [2048(16k), 96] bf16 from (96, 2048)
                txp = wload.tile([96, D], F32, tag="w2", bufs=1)
                nc.sync.dma_start(txp[:], xproj_d)
                for k in range(16):
                    pt = ps_tr.tile([P, 512], F32, tag="tr")
                    nc.tensor.transpose(
                        pt[:, 0:96], txp[:, k * P:(k + 1) * P], ident[0:96, 0:96]
                    )
                    nc.scalar.copy(xpT[:, k * 96:(k + 1) * 96], pt[:, 0:96])

                # dt_projT: [64, 2048] bf16 from (2048, 64)
                for k in range(16):
                    t = wload.tile([P, DTR], F32, tag="w")
                    nc.sync.dma_start(t[:], dtw_d[k * P:(k + 1) * P, :])
                    pt = ps_tr.tile([P, 512], F32, tag="tr")
                    nc.tensor.transpose(pt[0:DTR, 0:P], t[:], ident[:])
                    nc.scalar.copy(dtpT[:, k * P:(k + 1) * P], pt[0:DTR, 0:P])

                # ---- h = x @ w_inp + b_inp, in [feat, b] bf16 ----
                hT = wtmp.tile([P, (DM // P) * BC], BF16)
                for m in range(DM // P):  # 8
                    pm = ps_mm.tile([P, BC], F32, tag="mm")
                    for k in range(4):
                        nc.tensor.matmul(
                            pm[:], w_inp[:, k * DM + m * P: k * DM + (m + 1) * P],
                            xT[:, k * BC:(k + 1) * BC],
                            start=(k == 0), stop=(k == 3),
                        )
                    nc.scalar.activation(
                        hT[:, m * BC:(m + 1) * BC], pm[:], AF.Identity,
                        bias=b_inp[:, m:m + 1], scale=1.0,
                    )

                # ---- xz = h @ in_proj.T -> xiT (m<16), szT=silu(z) ----
                for m in range(32):
                    pm = ps_mm.tile([P, BC], F32, tag="mm")
                    for k in range(8):
                        nc.tensor.matmul(
                            pm[:], inT[:, k * 2 * D + m * P: k * 2 * D + (m + 1) * P],
                            hT[:, k * BC:(k + 1) * BC],
                            start=(k == 0), stop=(k == 7),
                        )
                    if m < 16:
                        nc.vector.tensor_copy(xiT[:, m * BC:(m + 1) * BC], pm[:])
                    else:
                        mm = m - 16
                        nc.scalar.activation(
                            szT[:, mm * BC:(mm + 1) * BC], pm[:], AF.Silu,
                            bias=0.0, scale=1.0,
                        )

            # ---- phase 1: conv over rnn (first read) + xdb accumulation ----
            with ExitStack() as ph0b:
                p1 = ph0b.enter_context(tc.tile_pool(name="p1", bufs=6))
                wdt = ph0b.enter_context(tc.tile_pool(name="wdt", bufs=1))

                pxdb = ps_xdb.tile([96, BC], F32)
                cwv = conv_w.rearrange("p (t j) -> p t j", j=DCV)
                for dblk in range(DBLK):
                    for bt in range(NBT):
                        in1 = p1.tile([P, P * T], F32, tag="in1")
                        nc.sync.dma_start(
                            in1[:],
                            rnn_d[bt * P:(bt + 1) * P,
                                  dblk * P * T:(dblk + 1) * P * T],
                        )
                        iv = in1[:].rearrange("p (d t) -> p d t", t=T)
                        # transpose cs_1..3 -> [d, b] into one psum tile
                        pt = ps_tr.tile([P, 512], F32, tag="tr")
                        for j in (1, 2, 3):
                            nc.tensor.transpose(
                                pt[:, (j - 1) * P: j * P], iv[:, :, j], ident[:]
                            )
                        tmp = p1.tile([P, P], F32, tag="cv")
                        nc.vector.tensor_scalar_mul(
                            tmp[:], pt[:, 0:P], cwv[:, dblk, 0:1]
                        )
                        nc.vector.scalar_tensor_tensor(
                            tmp[:], pt[:, P:2 * P], cwv[:, dblk, 1:2], tmp[:],
                            op0=ALU.mult, op1=ALU.add,
                        )
                        nc.vector.scalar_tensor_tensor(
                            tmp[:], pt[:, 2 * P:3 * P], cwv[:, dblk, 2:3], tmp[:],
                            op0=ALU.mult, op1=ALU.add,
                        )
                        nc.vector.scalar_tensor_tensor(
                            tmp[:],
                            xiT[:, dblk * BC + bt * P: dblk * BC + (bt + 1) * P],
                            cwv[:, dblk, 3:4], tmp[:], op0=ALU.mult, op1=ALU.add,
                        )
                        nc.scalar.activation(
                            xcT[:, dblk * BC + bt * P: dblk * BC + (bt + 1) * P],
                            tmp[:], AF.Silu, bias=conv_b[:, dblk:dblk + 1],
                            scale=1.0,
                        )
                    # accumulate xdb += xpT[dblk].T @ xcT[dblk]
                    nc.tensor.matmul(
                        pxdb[:], xpT[:, dblk * 96:(dblk + 1) * 96],
                        xcT[:, dblk * BC:(dblk + 1) * BC],
                        start=(dblk == 0), stop=(dblk == DBLK - 1),
                    )

                # ---- xdb split: dt-rank rows + Bm/Cm transposed per btile ----
                nc.vector.tensor_copy(xdb_dt[:], pxdb[0:DTR, :])
                nc.vector.tensor_copy(bmcm_sb[:], pxdb[DTR:96, :])
                for bt in range(NBT):
                    pt = ps_tr.tile([P, 512], F32, tag="tr")
                    nc.tensor.transpose(
                        pt[:, 0:2 * NST], bmcm_sb[:, bt * P:(bt + 1) * P],
                        ident[0:2 * NST, 0:2 * NST],
                    )
                    nc.vector.tensor_copy(bmcm[bt], pt[:, 0:2 * NST])

                # ---- dt = softplus(xdb_dt @ dt_proj.T + dt_proj_b) ----
                dtT = wdt.tile([P, DBLK * BC], F32)
                dtxcT = wdt.tile([P, DBLK * BC], F32)
                for m in range(DBLK):
                    pm = ps_mm.tile([P, BC], F32, tag="mm")
                    nc.tensor.matmul(
                        pm[:], dtpT[:, m * P:(m + 1) * P], xdb_dt[:],
                        start=True, stop=True,
                    )
                    te = p1.tile([P, BC], F32, tag="sp")
                    nc.scalar.activation(
                        te[:], pm[:], AF.Exp, bias=dtb_col[:, m:m + 1], scale=1.0
                    )
                    nc.scalar.activation(
                        dtT[:, m * BC:(m + 1) * BC], te[:], AF.Ln,
                        bias=1.0, scale=1.0,
                    )
                # dtxcT = dt * xc
                nc.vector.tensor_tensor(dtxcT[:], dtT[:], xcT[:], op=ALU.mult)

                # ---- transpose dtT, dtxcT to [b, d] (groups of 4 dblks) ----
                for bt in range(NBT):
                    for g in range(4):
                        pt = ps_tr.tile([P, 512], F32, tag="tr")
                        pt2 = ps_tr.tile([P, 512], F32, tag="tr")
                        for i in range(4):
                            dblk = g * 4 + i
                            nc.tensor.transpose(
                                pt[:, i * P:(i + 1) * P],
                                dtT[:, dblk * BC + bt * P: dblk * BC + (bt + 1) * P],
                                ident[:],
                            )
                            nc.tensor.transpose(
                                pt2[:, i * P:(i + 1) * P],
                                dtxcT[:, dblk * BC + bt * P: dblk * BC + (bt + 1) * P],
                                ident[:],
                            )
                        nc.vector.tensor_copy(
                            dt_b[bt][:, g * 512:(g + 1) * 512], pt[:]
                        )
                        nc.scalar.copy(
                            dtxc_b[bt][:, g * 512:(g + 1) * 512], pt2[:]
                        )

        # ---- prefetch phase-3 weights (lands during phase 2) ----
        p3w = top.enter_context(tc.tile_pool(name="p3w", bufs=1))
        opT = p3w.tile([P, 16 * DM], BF16)
        w_outp = p3w.tile([P, 8 * NOUT], BF16)
        with ExitStack() as phw:
            p3wl = phw.enter_context(tc.tile_pool(name="p3wl", bufs=6))
            for rg in range(2):
                tiles = []
                for rr in range(4):
                    t = p3wl.tile([P, D], F32, tag="wop", bufs=6)
                    nc.sync.dma_start(
                        t[:], outproj_d[(rg * 4 + rr) * P:(rg * 4 + rr + 1) * P, :]
                    )
                    tiles.append(t)
                for k in range(16):
                    pt = ps_tr.tile([P, 512], F32, tag="tr")
                    for rr in range(4):
                        nc.tensor.transpose(
                            pt[:, rr * P:(rr + 1) * P],
                            tiles[rr][:, k * P:(k + 1) * P], ident[:]
                        )
                    dst = opT[:, k * DM + rg * 512: k * DM + (rg + 1) * 512]
                    if k % 2 == 0:
                        nc.scalar.copy(dst, pt[:])
                    else:
                        nc.vector.tensor_copy(dst, pt[:])
            # w_outp natural [K=1024, M=512] -> bf16
            for k in range(8):
                t = p3wl.tile([P, NOUT], F32, tag="wo2")
                nc.sync.dma_start(t[:], w_outp_d[k * P:(k + 1) * P, :])
                nc.vector.tensor_copy(w_outp[:, k * NOUT:(k + 1) * NOUT], t[:])

        # ================= phase 2 + 3: SSM update, then y->out per btile =====
        with ExitStack() as ph2:
            p2 = ph2.enter_context(tc.tile_pool(name="p2", bufs=2))
            p2o = ph2.enter_context(tc.tile_pool(name="p2o", bufs=2))
            p2t = ph2.enter_context(tc.tile_pool(name="p2t", bufs=3))
            p2da = ph2.enter_context(tc.tile_pool(name="p2da", bufs=2))
            p2a = ph2.enter_context(tc.tile_pool(name="p2a", bufs=2))
            p3s = ph2.enter_context(tc.tile_pool(name="p3s", bufs=1))

            for bt in range(NBT):
                for ch in range(NCH):
                    unit = bt * NCH + ch
                    # A_rep = exp(A_log) replicated across partitions, bf16
                    a_rep = p2a.tile([P, DC2 * NST], BF16, tag="arep")
                    for g in range(DC2 * NST // 512):  # 4
                        pa = ps_tr.tile([P, 512], F32, tag="tr")
                        off = ch * DC2 * NST + g * 512
                        row, col = off // D, off % D
                        sel = ident[0:NST, row:row + 1].broadcast_to([NST, P])
                        nc.tensor.matmul(
                            pa[:], sel, arow16[:, col:col + 512],
                            start=True, stop=True,
                        )
                        nc.scalar.activation(
                            a_rep[:, g * 512:(g + 1) * 512], pa[:], AF.Exp,
                            bias=0.0, scale=1.0,
                        )
                    a_rep_v = a_rep[:].rearrange("p (d n) -> p d n", n=NST)

                    in2 = p2.tile([P, DC2 * T], F32, tag="in2")
                    nc.sync.dma_start(
                        in2[:],
                        rnn_d[bt * P:(bt + 1) * P, ch * DC2 * T:(ch + 1) * DC2 * T],
                    )
                    out2 = p2o.tile([P, DC2 * T], F32, tag="out2")
                    iv = in2[:].rearrange("p (d t) -> p d t", t=T)
                    ov = out2[:].rearrange("p (d t) -> p d t", t=T)

                    dts = dt_b[bt][:, ch * DC2:(ch + 1) * DC2]
                    dts_b = dts.unsqueeze(2).broadcast_to([P, DC2, NST])

                    prod = p2t.tile([P, DC2 * NST], F32, tag="t", bufs=2)
                    pv = prod[:].rearrange("p (d n) -> p d n", n=NST)
                    # balance the two big TT multiplies across DVE and Pool
                    phase = unit % 4
                    if _POOL_MODE == 0:
                        prod_eng, m_eng = nc.vector, nc.vector
                    elif _POOL_MODE == 1:
                        prod_eng, m_eng = nc.gpsimd, nc.vector
                    elif _POOL_MODE == 3:
                        prod_eng, m_eng = nc.vector, nc.gpsimd
                    else:  # mixed balance
                        prod_eng = nc.gpsimd if phase in (0, 1) else nc.vector
                        m_eng = nc.gpsimd if phase == 2 else nc.vector
                    prod_eng.tensor_tensor(pv, dts_b, a_rep_v, op=ALU.mult)

                    dA = p2da.tile([P, DC2 * NST], BF16, tag="da")
                    nc.scalar.activation(dA[:], prod[:], AF.Exp, bias=0.0, scale=-1.0)

                    # m = s_old * dA written straight into out2 state cols;
                    # alternate DVE/Pool to balance engines
                    dAv = dA[:].rearrange("p (d n) -> p d n", n=NST)
                    m_eng.tensor_tensor(
                        ov[:, :, DCV:T], iv[:, :, DCV:T], dAv, op=ALU.mult
                    )

                    # conv window shift: out cols 0..2 = in cols 1..3
                    nc.gpsimd.tensor_copy(ov[:, :, 0:DCV - 1], iv[:, :, 1:DCV])
                    # out col 3 = xi (transposed from xiT for this d-block)
                    pt = ps_tr.tile([P, 512], BF16, name="ptxi", tag="xi", bufs=1)
                    nc.tensor.transpose(
                        pt[:, 0:P],
                        xiT[:, ch * BC + bt * P: ch * BC + (bt + 1) * P],
                        ident_bf[:],
                    )
                    nc.scalar.copy(ov[:, :, DCV - 1], pt[:, 0:P])

                    dtxcs = dtxc_b[bt][:, ch * DC2:(ch + 1) * DC2]
                    ys = y_b[bt][:, ch * DC2:(ch + 1) * DC2]
                    # s_new = m + dtxc*Bm, in place in out2
                    for n in range(NST):
                        nc.vector.scalar_tensor_tensor(
                            ov[:, :, DCV + n], dtxcs, bmcm[bt][:, n:n + 1],
                            ov[:, :, DCV + n], op0=ALU.mult, op1=ALU.add,
                        )
                    for n in range(NST):
                        if n == 0:
                            nc.vector.tensor_scalar_mul(
                                ys, ov[:, :, DCV], bmcm[bt][:, NST:NST + 1]
                            )
                        else:
                            nc.vector.scalar_tensor_tensor(
                                ys, ov[:, :, DCV + n], bmcm[bt][:, NST + n:NST + n + 1],
                                ys, op0=ALU.mult, op1=ALU.add,
                            )
                    nc.sync.dma_start(
                        rnnout_d[bt * P:(bt + 1) * P, ch * DC2 * T:(ch + 1) * DC2 * T],
                        out2[:],
                    )

                # ---- phase 3 for this btile (overlaps next btile's phase 2) ----
                yT = p3s.tile([P, DBLK * P], BF16, tag="yT", bufs=1)
                for dblk in range(DBLK):
                    pt = ps_tr.tile([P, 512], F32, tag="tr")
                    nc.tensor.transpose(
                        pt[:, 0:P], y_b[bt][:, dblk * P:(dblk + 1) * P], ident[:]
                    )
                    nc.vector.scalar_tensor_tensor(
                        yT[:, dblk * P:(dblk + 1) * P],
                        xcT[:, dblk * BC + bt * P: dblk * BC + (bt + 1) * P],
                        d_vec[:, dblk:dblk + 1], pt[:, 0:P],
                        op0=ALU.mult, op1=ALU.add,
                    )
                yfT = p3s.tile([P, DBLK * P], BF16, tag="yfT", bufs=1)
                yfv = yfT[:].rearrange("p (k b) -> p k b", b=P)
                nc.vector.tensor_tensor(
                    yfv,
                    yT[:].rearrange("p (k b) -> p k b", b=P),
                    szT[:].rearrange("p (k b) -> p k b", b=BC)[:, :, bt * P:(bt + 1) * P],
                    op=ALU.mult,
                )

                # core_out = yf @ out_proj.T -> [1024(8m), 128] bf16
                coT = p3s.tile([P, 8 * P], BF16, tag="coT", bufs=1)
                for m in range(8):
                    pm = ps_mm.tile([P, P], F32, tag="mm")
                    for k in range(16):
                        nc.tensor.matmul(
                            pm[:], opT[:, k * DM + m * P: m * P + k * DM + P],
                            yfT[:, k * P:(k + 1) * P],
                            start=(k == 0), stop=(k == 15),
                        )
                    nc.scalar.copy(coT[:, m * P:(m + 1) * P], pm[:])

                # out = core_out @ w_outp + b_outp -> [512(4m), 128] f32
                outT = p3s.tile([P, 4 * P], F32, tag="outT", bufs=1)
                for m in range(4):
                    pm = ps_mm.tile([P, P], F32, tag="mm")
                    for k in range(8):
                        nc.tensor.matmul(
                            pm[:], w_outp[:, k * NOUT + m * P: m * P + k * NOUT + P],
                            coT[:, k * P:(k + 1) * P],
                            start=(k == 0), stop=(k == 7),
                        )
                    nc.scalar.activation(
                        outT[:, m * P:(m + 1) * P], pm[:], AF.Identity,
                        bias=b_outp[:, m:m + 1], scale=1.0,
                    )

                # transpose outT -> [b, 512] and store
                ob = p3s.tile([P, NOUT], F32, tag="ob", bufs=1)
                pt = ps_tr.tile([P, 512], F32, tag="tr")
                for m in range(4):
                    nc.tensor.transpose(
                        pt[:, m * P:(m + 1) * P], outT[:, m * P:(m + 1) * P], ident[:]
                    )
                nc.vector.tensor_copy(ob[:], pt[:])
                nc.sync.dma_start(out_d[bt * P:(bt + 1) * P, :], ob[:])

    nc.compile()
    return nc


def _get_program():
    if "nc" not in _CACHE:
        _CACHE["nc"] = _build()
    return _CACHE["nc"]


def kernel(**inputs):
    from concourse.bass_utils import run_bass_kernel_spmd

    nc = _get_program()

    x = np.asarray(inputs["x"], dtype=np.float32)             # (1, B, 512)
    rnn = np.asarray(inputs["rnn_states"], dtype=np.float32)  # (1, B, 40960)
    weights = {
        "w_inp": inputs["w_inp"], "b_inp": inputs["b_inp"],
        "w_outp": inputs["w_outp"], "b_outp": inputs["b_outp"],
        "in_proj_w": inputs["in_proj_w"], "conv_w": inputs["conv_w"],
        "conv_b": inputs["conv_b"], "x_proj_w": inputs["x_proj_w"],
        "dt_proj_w": inputs["dt_proj_w"], "dt_proj_b": inputs["dt_proj_b"],
        "A_log": inputs["A_log"], "D_vec": inputs["D"],
        "out_proj_w": inputs["out_proj_w"],
    }
    weights = {k: np.ascontiguousarray(np.asarray(v, dtype=np.float32))
               for k, v in weights.items()}

    in_maps = []
    for c in range(NCORES):
        sl = slice(c * BC, (c + 1) * BC)
        m = dict(weights)
        m["x_c"] = np.ascontiguousarray(x[0, sl])
        m["rnn_c"] = np.ascontiguousarray(rnn[0, sl])
        in_maps.append(m)

    res = run_bass_kernel_spmd(nc, in_maps, list(range(NCORES))).results

    out = np.empty((1, B, NOUT), dtype=np.float32)
    new_rnn = np.empty((B, D * T), dtype=np.float32)
    for c in range(NCORES):
        sl = slice(c * BC, (c + 1) * BC)
        out[0, sl] = res[c]["out_c"]
        new_rnn[sl] = res[c]["rnn_out_c"]
    return out, new_rnn
